# revision 6
# baseline (speedup 1.0000x reference)
"""MiniMax-style MoE layer (sigmoid gate, top-8 of 64 experts, SwiGLU FFN)
as an expert-parallel Bass kernel on 8 Trainium2 NeuronCores.

Sharding strategy (per the expert-parallel hint):
  * Host computes the (cheap) gate routing: logits -> sigmoid -> top-8 ->
    combine weights.  This is the "dispatch" step of expert parallelism and
    determines the input sharding.
  * The 64 experts are ranked by routed-token count and packed into
    8 slots x 8 cores so that each slot's capacity (compile-time constant,
    shared across cores by SPMD) is the max token count within that slot.
    Sorted packing makes sum(capacity) very close to sum(n_e)/8 per core.
  * Each core runs the same Bass program: for each of its 8 expert slots it
    computes  H^T = silu(Wg^T X^T) * (Wu^T X^T)  (bf16 matmuls, fp32 PSUM
    accumulation), then Y = H @ Wd, scales rows by the per-token combine
    weight and writes Y to HBM.
  * Host scatter-adds the per-slot outputs back into the [T, D] output
    (the "combine" step).
"""

import math

import ml_dtypes
import numpy as np

import concourse.bass as bass
import concourse.mybir as mybir
from concourse.bass_utils import run_bass_kernel_spmd
from concourse.tile import TileContext

B, S, D, F, E, TOP_K = 2, 2048, 2048, 1024, 64, 8
ROUTED_SCALING = 1.0
NCORES = 8
SLOTS = E // NCORES
P = 128

BF16 = ml_dtypes.bfloat16
LAST_C_LIST = None  # slot capacities used by the most recent moe_forward call


def _chunk_bounds(C, max_chunk=512):
    """Split [0, C) into ceil(C/max_chunk) nearly-even chunks (8-aligned)."""
    n = max(1, math.ceil(C / max_chunk))
    step = ((C + n - 1) // n + 7) // 8 * 8
    bounds = []
    c0 = 0
    while c0 < C:
        c1 = min(C, c0 + step)
        bounds.append((c0, c1))
        c0 = c1
    return bounds


def _split_sync_waits(nc, limit=1):
    """This walrus build encodes at most one sync-wait per instruction.

    Tile emits multi-wait sync_info (e.g. the kernel-tail drain waits on
    every engine + DMA queue); split the excess waits onto dedicated
    single-wait no-ops placed just before the instruction on the same
    engine (AND semantics are preserved by sequential waits).
    """
    idx = 0
    for fn in nc.m.functions:
        for bb in fn.blocks:
            insts = bb.instructions
            i = 0
            while i < len(insts):
                inst = insts[i]
                si = getattr(inst, "sync_info", None)
                if si is not None and si.on_wait and len(si.on_wait) > limit:
                    waits = list(si.on_wait)
                    pre = []
                    while len(waits) > limit:
                        chunk, waits = waits[:limit], waits[limit:]
                        nop = mybir.InstNoOp(
                            name=f"waitsplit-{idx}",
                            engine=inst.engine,
                            sync_info=mybir.SyncInfo(on_wait=chunk, on_update=[]),
                            bass_nofuse=True,
                        )
                        nc.register_instruction(nop, overwrite=True)
                        idx += 1
                        pre.append(nop)
                    si.on_wait = waits
                    insts[i:i] = pre
                    i += len(pre)
                i += 1


def build_nc(C_list, d=D, f=F, trace_scopes=False):
    """Build the SPMD per-core Bass program for slot capacities C_list."""
    ko_d = d // P          # contraction tiles for gate/up
    ft_n = f // P          # output row-tiles of H^T
    ko_f = f // P          # contraction tiles for down
    dc_n = d // 512        # output column chunks of Y
    bf = mybir.dt.bfloat16
    f32 = mybir.dt.float32

    nc = bass.Bass()
    xts, wgs, wus, wds, cws, ys = [], [], [], [], [], []
    for s, C in enumerate(C_list):
        MT = math.ceil(C / P)
        xts.append(nc.dram_tensor(f"xt{s}", [P, ko_d, C], bf, kind="ExternalInput"))
        wgs.append(nc.dram_tensor(f"wg{s}", [ft_n, P, ko_d, P], bf, kind="ExternalInput"))
        wus.append(nc.dram_tensor(f"wu{s}", [ft_n, P, ko_d, P], bf, kind="ExternalInput"))
        wds.append(nc.dram_tensor(f"wd{s}", [ko_f, P, d], bf, kind="ExternalInput"))
        cws.append(nc.dram_tensor(f"cw{s}", [P, MT], f32, kind="ExternalInput"))
        ys.append(nc.dram_tensor(f"y{s}", [MT * P, d], f32, kind="ExternalOutput"))

    with TileContext(nc) as tc:
        with (
            tc.tile_pool(name="xt", bufs=2) as xt_pool,
            tc.tile_pool(name="w", bufs=3) as w_pool,
            tc.tile_pool(name="wd", bufs=2) as wd_pool,
            tc.tile_pool(name="ht", bufs=2) as ht_pool,
            tc.tile_pool(name="tmp", bufs=3) as tmp_pool,
            tc.tile_pool(name="out", bufs=4) as out_pool,
            tc.tile_pool(name="cw", bufs=2) as cw_pool,
            tc.tile_pool(name="psg", bufs=2, space="PSUM") as psg_pool,
            tc.tile_pool(name="psu", bufs=2, space="PSUM") as psu_pool,
            tc.tile_pool(name="psy", bufs=3, space="PSUM") as psy_pool,
        ):
            for s, C in enumerate(C_list):
                MT = math.ceil(C / P)
                chunks = _chunk_bounds(C)

                xt_t = xt_pool.tile([P, ko_d, C], bf, tag="xt")
                nc.sync.dma_start(xt_t[:], xts[s][:])
                cw_t = cw_pool.tile([P, MT], f32, tag="cw")
                nc.sync.dma_start(cw_t[:], cws[s][:])
                ht_t = ht_pool.tile([P, ko_f, C], bf, tag="ht")

                for ft in range(ft_n):
                    wg_t = w_pool.tile([P, ko_d, P], bf, tag="wg")
                    nc.sync.dma_start(wg_t[:], wgs[s][ft])
                    wu_t = w_pool.tile([P, ko_d, P], bf, tag="wu")
                    nc.sync.dma_start(wu_t[:], wus[s][ft])
                    for (c0, c1) in chunks:
                        w_ = c1 - c0
                        pg = psg_pool.tile([P, 512], f32, tag="pg", name="pg")[:, :w_]
                        for ko in range(ko_d):
                            nc.tensor.matmul(
                                pg, wg_t[:, ko], xt_t[:, ko, c0:c1],
                                start=(ko == 0), stop=(ko == ko_d - 1),
                            )
                        sg = tmp_pool.tile([P, 512], f32, tag="sg", name="sg")[:, :w_]
                        nc.scalar.activation(sg, pg, mybir.ActivationFunctionType.Silu)
                        pu = psu_pool.tile([P, 512], f32, tag="pu", name="pu")[:, :w_]
                        for ko in range(ko_d):
                            nc.tensor.matmul(
                                pu, wu_t[:, ko], xt_t[:, ko, c0:c1],
                                start=(ko == 0), stop=(ko == ko_d - 1),
                            )
                        nc.vector.tensor_mul(ht_t[:, ft, c0:c1], sg, pu)

                wd_t = wd_pool.tile([P, ko_f, d], bf, tag="wd")
                for kt in range(ko_f):
                    nc.sync.dma_start(wd_t[:, kt], wds[s][kt])

                for m in range(MT):
                    rem = min(P, C - m * P)
                    for dc in range(dc_n):
                        py = psy_pool.tile([P, 512], f32, tag="py", name="py")[:rem]
                        for kt in range(ko_f):
                            nc.tensor.matmul(
                                py,
                                ht_t[:, kt, m * P : m * P + rem],
                                wd_t[:, kt, dc * 512 : (dc + 1) * 512],
                                start=(kt == 0), stop=(kt == ko_f - 1),
                            )
                        yt = out_pool.tile([P, 512], f32, tag="y", name="yt")[:rem]
                        nc.vector.tensor_scalar_mul(yt, py, cw_t[:rem, m : m + 1])
                        nc.sync.dma_start(
                            ys[s][m * P : m * P + rem, dc * 512 : (dc + 1) * 512], yt
                        )
    _split_sync_waits(nc)
    return nc


def route(x2d, gate_w, e_bias):
    """Replicate the reference routing on host (fp32).

    Returns (expert token lists, per-pair combine weights, counts).
    """
    T = x2d.shape[0]
    logits = x2d @ gate_w.T.astype(np.float32)             # [T, E]
    scores = 1.0 / (1.0 + np.exp(-logits))
    biased = scores + e_bias[None, :].astype(np.float32)
    # jax.lax.top_k: k largest, ties -> lower index first.
    topk_idx = np.argsort(-biased, axis=-1, kind="stable")[:, :TOP_K]
    topk_scores = np.take_along_axis(scores, topk_idx, axis=-1)
    topk_w = topk_scores / (topk_scores.sum(-1, keepdims=True) + 1e-20)
    topk_w = topk_w * ROUTED_SCALING

    flat_e = topk_idx.ravel()
    pair_tok = np.repeat(np.arange(T, dtype=np.int64), TOP_K)
    pair_w = topk_w.ravel()
    order = np.argsort(flat_e, kind="stable")
    counts = np.bincount(flat_e, minlength=E)
    starts = np.concatenate([[0], np.cumsum(counts)])
    toks = [pair_tok[order[starts[e] : starts[e + 1]]] for e in range(E)]
    ws = [pair_w[order[starts[e] : starts[e + 1]]] for e in range(E)]
    return toks, ws, counts


def pack_experts(counts):
    """Assign experts to (core, slot); returns assignment and capacities."""
    rank = np.argsort(-counts, kind="stable")
    assign = {}  # (core, slot) -> expert id
    C_list = []
    for s in range(SLOTS):
        grp = rank[s * NCORES : (s + 1) * NCORES]
        cap = max(8, int(counts[grp].max()))
        C_list.append(cap)
        for c, e in enumerate(grp):
            assign[(c, s)] = int(e)
    return assign, C_list


def _prep_core_inputs(core, assign, C_list, x2d, toks, ws, Wg_b, Wu_b, Wd_b,
                      d=D, f=F):
    ko_d = d // P
    ft_n = f // P
    ko_f = f // P
    in_map = {}
    for s, C in enumerate(C_list):
        e = assign[(core, s)]
        tok = toks[e]
        n = len(tok)
        MT = math.ceil(C / P)

        xt = np.zeros((P, ko_d, C), dtype=BF16)
        if n:
            g = x2d[tok].astype(BF16)                 # [n, d]
            # [n, d] -> [d, n] -> [ko, P, n] -> [P, ko, n]
            xt[:, :, :n] = np.ascontiguousarray(
                g.T.reshape(ko_d, P, n).transpose(1, 0, 2)
            )
        in_map[f"xt{s}"] = xt

        # Wg/Wu [d, f] -> [ft, P(ki), ko, P(fi)]
        wg = Wg_b[e].reshape(ko_d, P, ft_n, P).transpose(2, 1, 0, 3)
        wu = Wu_b[e].reshape(ko_d, P, ft_n, P).transpose(2, 1, 0, 3)
        in_map[f"wg{s}"] = np.ascontiguousarray(wg)
        in_map[f"wu{s}"] = np.ascontiguousarray(wu)
        # Wd [f, d] -> [ko_f, P, d]
        in_map[f"wd{s}"] = Wd_b[e].reshape(ko_f, P, d)

        cw = np.zeros((MT * P,), dtype=np.float32)
        if n:
            cw[:n] = ws[e]
        in_map[f"cw{s}"] = np.ascontiguousarray(cw.reshape(MT, P).T)
    return in_map


def moe_forward(x, gate_w, e_bias, Wg, Wu, Wd, trace=False):
    b, s_len, d = x.shape
    f = Wg.shape[2]
    T = b * s_len
    x2d = np.asarray(x, dtype=np.float32).reshape(T, d)

    toks, ws, counts = route(x2d, np.asarray(gate_w), np.asarray(e_bias))
    assign, C_list = pack_experts(counts)
    global LAST_C_LIST
    LAST_C_LIST = C_list

    nc = build_nc(tuple(C_list), d=d, f=f)

    Wg_b = np.asarray(Wg).astype(BF16)
    Wu_b = np.asarray(Wu).astype(BF16)
    Wd_b = np.asarray(Wd).astype(BF16)

    in_maps = [
        _prep_core_inputs(c, assign, C_list, x2d, toks, ws, Wg_b, Wu_b, Wd_b,
                          d=d, f=f)
        for c in range(NCORES)
    ]

    res = run_bass_kernel_spmd(nc, in_maps, list(range(NCORES)), trace=trace)

    out = np.zeros((T, d), dtype=np.float32)
    for c in range(NCORES):
        for s in range(SLOTS):
            e = assign[(c, s)]
            n = len(toks[e])
            if n:
                out[toks[e]] += res.results[c][f"y{s}"][:n]
    return out.reshape(b, s_len, d), res


def kernel(x, gate_w, e_bias, Wg, Wu, Wd):
    out, _ = moe_forward(x, gate_w, e_bias, Wg, Wu, Wd)
    return out


# revision 12
# speedup vs baseline: 8.9011x; 8.9011x over previous
"""MiniMax-style MoE layer (sigmoid gate, top-8 of 64 experts, SwiGLU FFN)
as an expert-parallel Bass kernel on 8 Trainium2 NeuronCores.

Sharding strategy (per the expert-parallel hint):
  * Host computes the (cheap) gate routing: logits -> sigmoid -> top-8 ->
    combine weights.  This is the "dispatch" step of expert parallelism and
    determines the input sharding.
  * The 64 experts are ranked by routed-token count and packed into
    8 slots x 8 cores so that each slot's capacity (compile-time constant,
    shared across cores by SPMD) is the max token count within that slot.
    Sorted packing makes sum(capacity) very close to sum(n_e)/8 per core.
  * Each core runs the same Bass program: for each of its 8 expert slots it
    computes  H^T = silu(Wg^T X^T) * (Wu^T X^T)  (bf16 matmuls, fp32 PSUM
    accumulation), then Y = H @ Wd, scales rows by the per-token combine
    weight and writes Y to HBM.
  * Host scatter-adds the per-slot outputs back into the [T, D] output
    (the "combine" step).
"""

import math

import ml_dtypes
import numpy as np

import concourse.bass as bass
import concourse.mybir as mybir
from concourse.bass_utils import run_bass_kernel_spmd
from concourse.tile import TileContext

B, S, D, F, E, TOP_K = 2, 2048, 2048, 1024, 64, 8
ROUTED_SCALING = 1.0
NCORES = 8
SLOTS = E // NCORES
P = 128

BF16 = ml_dtypes.bfloat16
LAST_C_LIST = None  # slot capacities used by the most recent moe_forward call


def _chunk_bounds(C, max_chunk=512):
    """Split [0, C) into ceil(C/max_chunk) nearly-even chunks (8-aligned)."""
    n = max(1, math.ceil(C / max_chunk))
    step = ((C + n - 1) // n + 7) // 8 * 8
    bounds = []
    c0 = 0
    while c0 < C:
        c1 = min(C, c0 + step)
        bounds.append((c0, c1))
        c0 = c1
    return bounds


def _split_sync_waits(nc, limit=1):
    """This walrus build encodes at most one sync-wait per instruction.

    Tile emits multi-wait sync_info (e.g. the kernel-tail drain waits on
    every engine + DMA queue); split the excess waits onto dedicated
    single-wait no-ops placed just before the instruction on the same
    engine (AND semantics are preserved by sequential waits).
    """
    idx = 0
    for fn in nc.m.functions:
        for bb in fn.blocks:
            insts = bb.instructions
            i = 0
            while i < len(insts):
                inst = insts[i]
                si = getattr(inst, "sync_info", None)
                if si is not None and si.on_wait and len(si.on_wait) > limit:
                    waits = list(si.on_wait)
                    pre = []
                    while len(waits) > limit:
                        chunk, waits = waits[:limit], waits[limit:]
                        nop = mybir.InstNoOp(
                            name=f"waitsplit-{idx}",
                            engine=inst.engine,
                            sync_info=mybir.SyncInfo(on_wait=chunk, on_update=[]),
                            bass_nofuse=True,
                        )
                        nc.register_instruction(nop, overwrite=True)
                        idx += 1
                        pre.append(nop)
                    si.on_wait = waits
                    insts[i:i] = pre
                    i += len(pre)
                i += 1


def build_nc(C_list, d=D, f=F, chain_io=False, reps=None):
    """Build the SPMD per-core Bass program for slot capacities C_list.

    chain_io adds a tiny pass-through input/output pair used only by the
    benchmark harness; reps wraps the body in a For_i loop (benchmark-only)
    so per-iteration device time can be measured as a slope.
    """
    ko_d = d // P          # contraction tiles for gate/up
    ft_n = f // P          # output row-tiles of H^T
    ko_f = f // P          # contraction tiles for down
    dc_n = d // 512        # output column chunks of Y
    bf = mybir.dt.bfloat16
    f32 = mybir.dt.float32

    nc = bass.Bass()
    ch_in = ch_out = None
    if chain_io:
        ch_in = nc.dram_tensor("chain", [P, 8], f32, kind="ExternalInput")
        ch_out = nc.dram_tensor("chain_out", [P, 8], f32, kind="ExternalOutput")
    xts, wgs, wus, wds, cws, ys = [], [], [], [], [], []
    for s, C in enumerate(C_list):
        MT = math.ceil(C / P)
        xts.append(nc.dram_tensor(f"xt{s}", [P, ko_d, C], bf, kind="ExternalInput"))
        wgs.append(nc.dram_tensor(f"wg{s}", [ft_n, P, ko_d, P], bf, kind="ExternalInput"))
        wus.append(nc.dram_tensor(f"wu{s}", [ft_n, P, ko_d, P], bf, kind="ExternalInput"))
        wds.append(nc.dram_tensor(f"wd{s}", [ko_f, P, d], bf, kind="ExternalInput"))
        cws.append(nc.dram_tensor(f"cw{s}", [P, MT], f32, kind="ExternalInput"))
        ys.append(nc.dram_tensor(f"y{s}", [MT * P, d], f32, kind="ExternalOutput"))

    with TileContext(nc) as tc:
        with (
            tc.tile_pool(name="xt", bufs=2) as xt_pool,
            tc.tile_pool(name="w", bufs=3) as w_pool,
            tc.tile_pool(name="wd", bufs=2) as wd_pool,
            tc.tile_pool(name="ht", bufs=2) as ht_pool,
            tc.tile_pool(name="tmp", bufs=3) as tmp_pool,
            tc.tile_pool(name="out", bufs=4) as out_pool,
            tc.tile_pool(name="cw", bufs=2) as cw_pool,
            tc.tile_pool(name="psg", bufs=2, space="PSUM") as psg_pool,
            tc.tile_pool(name="psu", bufs=2, space="PSUM") as psu_pool,
            tc.tile_pool(name="psy", bufs=3, space="PSUM") as psy_pool,
        ):
            import contextlib
            loop_cm = tc.For_i(0, reps, 1) if reps else contextlib.nullcontext()
            with loop_cm:
              for s, C in enumerate(C_list):
                MT = math.ceil(C / P)
                chunks = _chunk_bounds(C)

                xt_t = xt_pool.tile([P, ko_d, C], bf, tag="xt")
                nc.sync.dma_start(xt_t[:], xts[s][:])
                cw_t = cw_pool.tile([P, MT], f32, tag="cw")
                nc.sync.dma_start(cw_t[:], cws[s][:])
                ht_t = ht_pool.tile([P, ko_f, C], bf, tag="ht")

                for ft in range(ft_n):
                    wg_t = w_pool.tile([P, ko_d, P], bf, tag="wg")
                    nc.sync.dma_start(wg_t[:], wgs[s][ft])
                    wu_t = w_pool.tile([P, ko_d, P], bf, tag="wu")
                    nc.sync.dma_start(wu_t[:], wus[s][ft])
                    for (c0, c1) in chunks:
                        w_ = c1 - c0
                        pg = psg_pool.tile([P, 512], f32, tag="pg", name="pg")[:, :w_]
                        for ko in range(ko_d):
                            nc.tensor.matmul(
                                pg, wg_t[:, ko], xt_t[:, ko, c0:c1],
                                start=(ko == 0), stop=(ko == ko_d - 1),
                            )
                        sg = tmp_pool.tile([P, 512], f32, tag="sg", name="sg")[:, :w_]
                        nc.scalar.activation(sg, pg, mybir.ActivationFunctionType.Silu)
                        pu = psu_pool.tile([P, 512], f32, tag="pu", name="pu")[:, :w_]
                        for ko in range(ko_d):
                            nc.tensor.matmul(
                                pu, wu_t[:, ko], xt_t[:, ko, c0:c1],
                                start=(ko == 0), stop=(ko == ko_d - 1),
                            )
                        nc.vector.tensor_mul(ht_t[:, ft, c0:c1], sg, pu)

                wd_t = wd_pool.tile([P, ko_f, d], bf, tag="wd")
                for kt in range(ko_f):
                    nc.sync.dma_start(wd_t[:, kt], wds[s][kt])

                for m in range(MT):
                    rem = min(P, C - m * P)
                    for dc in range(dc_n):
                        py = psy_pool.tile([P, 512], f32, tag="py", name="py")[:rem]
                        for kt in range(ko_f):
                            nc.tensor.matmul(
                                py,
                                ht_t[:, kt, m * P : m * P + rem],
                                wd_t[:, kt, dc * 512 : (dc + 1) * 512],
                                start=(kt == 0), stop=(kt == ko_f - 1),
                            )
                        yt = out_pool.tile([P, 512], f32, tag="y", name="yt")[:rem]
                        nc.vector.tensor_scalar_mul(yt, py, cw_t[:rem, m : m + 1])
                        nc.sync.dma_start(
                            ys[s][m * P : m * P + rem, dc * 512 : (dc + 1) * 512], yt
                        )
            if chain_io:
                cht = cw_pool.tile([P, 8], f32, tag="chain")
                nc.sync.dma_start(cht[:], ch_in[:])
                nc.sync.dma_start(ch_out[:], cht[:])
    _split_sync_waits(nc)
    return nc


def route(x2d, gate_w, e_bias):
    """Replicate the reference routing on host (fp32).

    Returns (expert token lists, per-pair combine weights, counts).
    """
    T = x2d.shape[0]
    logits = x2d @ gate_w.T.astype(np.float32)             # [T, E]
    scores = 1.0 / (1.0 + np.exp(-logits))
    biased = scores + e_bias[None, :].astype(np.float32)
    # jax.lax.top_k: k largest, ties -> lower index first.
    topk_idx = np.argsort(-biased, axis=-1, kind="stable")[:, :TOP_K]
    topk_scores = np.take_along_axis(scores, topk_idx, axis=-1)
    topk_w = topk_scores / (topk_scores.sum(-1, keepdims=True) + 1e-20)
    topk_w = topk_w * ROUTED_SCALING

    flat_e = topk_idx.ravel()
    pair_tok = np.repeat(np.arange(T, dtype=np.int64), TOP_K)
    pair_w = topk_w.ravel()
    order = np.argsort(flat_e, kind="stable")
    counts = np.bincount(flat_e, minlength=E)
    starts = np.concatenate([[0], np.cumsum(counts)])
    toks = [pair_tok[order[starts[e] : starts[e + 1]]] for e in range(E)]
    ws = [pair_w[order[starts[e] : starts[e + 1]]] for e in range(E)]
    return toks, ws, counts


def pack_experts(counts):
    """Assign experts to (core, slot); returns assignment and capacities."""
    rank = np.argsort(-counts, kind="stable")
    assign = {}  # (core, slot) -> expert id
    C_list = []
    for s in range(SLOTS):
        grp = rank[s * NCORES : (s + 1) * NCORES]
        cap = max(8, int(counts[grp].max()))
        C_list.append(cap)
        for c, e in enumerate(grp):
            assign[(c, s)] = int(e)
    return assign, C_list


def _prep_core_inputs(core, assign, C_list, x2d, toks, ws, Wg_b, Wu_b, Wd_b,
                      d=D, f=F):
    ko_d = d // P
    ft_n = f // P
    ko_f = f // P
    in_map = {}
    for s, C in enumerate(C_list):
        e = assign[(core, s)]
        tok = toks[e]
        n = len(tok)
        MT = math.ceil(C / P)

        xt = np.zeros((P, ko_d, C), dtype=BF16)
        if n:
            g = x2d[tok].astype(BF16)                 # [n, d]
            # [n, d] -> [d, n] -> [ko, P, n] -> [P, ko, n]
            xt[:, :, :n] = np.ascontiguousarray(
                g.T.reshape(ko_d, P, n).transpose(1, 0, 2)
            )
        in_map[f"xt{s}"] = xt

        # Wg/Wu [d, f] -> [ft, P(ki), ko, P(fi)]
        wg = Wg_b[e].reshape(ko_d, P, ft_n, P).transpose(2, 1, 0, 3)
        wu = Wu_b[e].reshape(ko_d, P, ft_n, P).transpose(2, 1, 0, 3)
        in_map[f"wg{s}"] = np.ascontiguousarray(wg)
        in_map[f"wu{s}"] = np.ascontiguousarray(wu)
        # Wd [f, d] -> [ko_f, P, d]
        in_map[f"wd{s}"] = Wd_b[e].reshape(ko_f, P, d)

        cw = np.zeros((MT * P,), dtype=np.float32)
        if n:
            cw[:n] = ws[e]
        in_map[f"cw{s}"] = np.ascontiguousarray(cw.reshape(MT, P).T)
    return in_map


def moe_forward(x, gate_w, e_bias, Wg, Wu, Wd, trace=False):
    b, s_len, d = x.shape
    f = Wg.shape[2]
    T = b * s_len
    x2d = np.asarray(x, dtype=np.float32).reshape(T, d)

    toks, ws, counts = route(x2d, np.asarray(gate_w), np.asarray(e_bias))
    assign, C_list = pack_experts(counts)
    global LAST_C_LIST
    LAST_C_LIST = C_list

    nc = build_nc(tuple(C_list), d=d, f=f)

    Wg_b = np.asarray(Wg).astype(BF16)
    Wu_b = np.asarray(Wu).astype(BF16)
    Wd_b = np.asarray(Wd).astype(BF16)

    in_maps = [
        _prep_core_inputs(c, assign, C_list, x2d, toks, ws, Wg_b, Wu_b, Wd_b,
                          d=d, f=f)
        for c in range(NCORES)
    ]

    res = run_bass_kernel_spmd(nc, in_maps, list(range(NCORES)), trace=trace)

    out = np.zeros((T, d), dtype=np.float32)
    for c in range(NCORES):
        for s in range(SLOTS):
            e = assign[(c, s)]
            n = len(toks[e])
            if n:
                out[toks[e]] += res.results[c][f"y{s}"][:n]
    return out.reshape(b, s_len, d), res


def kernel(x, gate_w, e_bias, Wg, Wu, Wd):
    out, _ = moe_forward(x, gate_w, e_bias, Wg, Wu, Wd)
    return out
